# revision 1
# baseline (speedup 1.0000x reference)
"""Trainium2 Bass kernel for nn_Attention_47493748359201.

Single-head attention: q/k/v projections -> softmax(q k^T) v -> output proj.
Full shapes: query/keys/values [4, 2048, 1024], weights [1024, 1024].

Sharding: 8 cores = (batch, query-half). Each core computes the full K/V
projection for its batch plus its own 1024-row query slice; no collectives.

Precision: bf16 hi/lo 3-pass matmuls for the q/k projections and the score
matmul (softmax is sensitive to absolute score error); fp32r (11-bit
mantissa on TRN2, measured) single-pass for the v projection, attend, and
output projection.

Bias handling (exact):
  - bk: the scores term q.bk is constant along the key axis -> drops out of
    softmax; bq.bk is a global constant -> drops too.  bk never ships.
  - bq: enters only through colbias[sk] = bq . k[sk] = keys[sk] . (Wk @ bq),
    computed exactly on the host (tiny matvec) and added to scores.
  - bv: softmax rows sum to 1, so attended += bv (per-partition add during
    the attend PSUM eviction).
  - bd: partition-broadcast add during the output PSUM eviction.
"""
import sys

sys.path.insert(0, "/opt/trn_rl_repo")

import numpy as np
import ml_dtypes

import concourse.bass as bass
import concourse.mybir as mybir
import concourse.tile as tile
from concourse import bacc
from concourse.masks import make_identity

P = 128
NB = 512  # matmul moving free dim (one PSUM bank of f32)
AF = mybir.ActivationFunctionType
ALU = mybir.AluOpType
dt = mybir.dt
f32 = dt.float32
f32r = dt.float32r
bf16 = dt.bfloat16
BF16 = ml_dtypes.bfloat16

# full-problem constants
B, S, D, H, DEP = 4, 2048, 1024, 1024, 1024
NCORES = 8
SQ = B * S // NCORES  # 1024 query rows per core


def input_specs(S=S, D=D, H=H, DEP=DEP, SQ=SQ, pair=False):
    """name -> (shape, mybir dtype) for the per-core DRAM inputs."""
    DT, HT = D // P, H // P
    SK = S // 2 if pair else S
    return {
        "xqh": ([P, DT, SQ], bf16), "xql": ([P, DT, SQ], bf16),
        "xkh": ([P, DT, SK], bf16), "xkl": ([P, DT, SK], bf16),
        "xv": ([P, DT, SK], f32r),
        "wqh": ([P, DT, H], bf16), "wql": ([P, DT, H], bf16),
        "wkh": ([P, DT, H], bf16), "wkl": ([P, DT, H], bf16),
        "wv": ([P, DT, H], f32r),
        "wd": ([P, HT, DEP], f32r),
        "bv": ([P, HT], f32),
        "bd": ([P, DEP], f32),
        "colbias": ([P, S], f32),
    }


def emit_attention(ctx, tc, io, S=S, D=D, H=H, DEP=DEP, SQ=SQ, fr=f32r,
                   pair=False):
    """Emit the per-core attention program. io: dict name -> bass.AP
    (input_specs() names plus "out" [SQ, DEP] f32)."""
    nc = tc.nc
    DT, HT, SKT, SQT = D // P, H // P, S // P, SQ // P
    SKC = S // NB          # score/key column chunks
    HC = H // NB           # h chunks
    DC = DEP // NB         # output dep chunks
    NBQ = min(NB, SQ)      # projection column chunk
    SH = min(2 * NB, S)    # score half-row size (<= 2 psum banks)
    NSH = S // SH          # halves per score row
    SQC = min(NB, SQ)      # attend rhs chunk
    NSQC = SQ // SQC
    H2 = max(H // 2, NB)   # v low/high split point
    assert H2 <= H

    # ---------------- resident SBUF (whole kernel) ----------------
    res = ctx.enter_context(tc.tile_pool(name="res", bufs=1))
    ident_f = res.tile([P, P], f32)
    make_identity(nc, ident_f[:])
    ident = res.tile([P, P], fr)
    nc.vector.tensor_copy(ident[:], ident_f[:])
    colbias = res.tile([P, S], f32)
    nc.sync.dma_start(colbias[:], io["colbias"])
    bv_t = res.tile([P, HT], f32)
    nc.sync.dma_start(bv_t[:], io["bv"])

    # big rotating slots (bufs=3): kth, ktl, qc -> vlo, vhi, attendedT
    big = ctx.enter_context(tc.tile_pool(name="big", bufs=3))
    kth = big.tile([P, HT, S], bf16, tag="big")
    ktl = big.tile([P, HT, S], bf16, tag="big")
    qc = big.tile([P, 2, HT, SQ], bf16, tag="big")
    qth = qc[:, 0]
    qtl = qc[:, 1]

    # DRAM scratch for v (natural [sk, h] layout, tiled [SKT, P, H])
    dram = ctx.enter_context(tc.tile_pool(name="dram", bufs=1, space="DRAM"))
    v_stage = dram.tile([SKT, P, H], fr)

    ps = ctx.enter_context(tc.tile_pool(name="ps", bufs=1, space="PSUM"))
    # stream pool: persistent tags -> DMA prefetch crosses phase boundaries
    strm = ctx.enter_context(tc.tile_pool(name="strm", bufs=1))

    # ---------------- phases 0-2: projections ----------------
    def load_x_chunk(xh_ap, xl_ap, c):
        cs = slice(c * NBQ, (c + 1) * NBQ)
        xh = strm.tile([P, DT, NBQ], bf16, name="xh", tag="xs", bufs=4)
        xl = strm.tile([P, DT, NBQ], bf16, name="xl", tag="xs", bufs=4)
        for do in range(DT):
            nc.sync.dma_start(xh[:, do, :], xh_ap[:, do, cs])
            nc.sync.dma_start(xl[:, do, :], xl_ap[:, do, cs])
        return xh, xl

    def proj_hilo(w_ab, xh_ap, xl_ap, outh, outl, ncols,
                  dram_out=None, first_x=None):
        """out[h, col] (hi/lo bf16) = W^T @ x, 3-pass hi/lo, no bias.
        dram_out: if set, evict hi/lo to dram_out[:, 0/1, ho, cs] instead of
        the SBUF tiles outh/outl."""
        for c in range(ncols // NBQ):
            cs = slice(c * NBQ, (c + 1) * NBQ)
            if c == 0 and first_x is not None:
                xh, xl = first_x
            else:
                xh, xl = load_x_chunk(xh_ap, xl_ap, c)
            for ho in range(HT):
                pt = ps.tile([P, NBQ], f32, tag="mm", name="pt", bufs=2)
                w_t = w_ab[ho // HT_W]
                hs = slice((ho % HT_W) * P, (ho % HT_W + 1) * P)
                for do in range(DT):
                    nc.tensor.matmul(pt[:], w_t[:, 0, do, hs], xh[:, do, :],
                                     start=(do == 0), stop=False)
                    nc.tensor.matmul(pt[:], w_t[:, 0, do, hs], xl[:, do, :],
                                     start=False, stop=False)
                    nc.tensor.matmul(pt[:], w_t[:, 1, do, hs], xh[:, do, :],
                                     start=False, stop=(do == DT - 1))
                if dram_out is None:
                    nc.scalar.activation(outh[:, ho, cs], pt[:], AF.Copy)
                    nc.vector.tensor_tensor(outl[:, ho, cs], pt[:],
                                            outh[:, ho, cs], ALU.subtract)
                else:
                    eh = strm.tile([P, NBQ], bf16, name="eh", tag="vo",
                                   bufs=2)
                    el = strm.tile([P, NBQ], bf16, name="el", tag="vo",
                                   bufs=2)
                    nc.scalar.activation(eh[:], pt[:], AF.Copy)
                    nc.vector.tensor_tensor(el[:], pt[:], eh[:],
                                            ALU.subtract)
                    nc.sync.dma_start(dram_out[:, 0, ho, cs], eh[:])
                    nc.sync.dma_start(dram_out[:, 1, ho, cs], el[:])

    SKO_OWN = SKT // 2 if pair else SKT   # own v row tiles
    SK_OWN = S // 2 if pair else S        # own key columns
    if pair:
        groups = [[2 * i, 2 * i + 1] for i in range(NCORES // 2)]
        k_own = dram.tile([P, 2, HT, SK_OWN], bf16, name="k_own")
        k_gath = dram.tile([2, P, 2, HT, SK_OWN], bf16, name="k_gath")
        v_own = dram.tile([SKO_OWN, P, H], fr, name="v_own")

    H2W = H // 2 if H // 2 >= NB else H
    NWH = H // H2W

    def load_whalf(wp, nm, hi_ap, lo_ap, half):
        t = wp.tile([P, 2, DT, H2W], bf16, name=nm, tag="w", bufs=3)
        hs = slice(half * H2W, (half + 1) * H2W)
        for do in range(DT):
            nc.sync.dma_start(t[:, 0, do, :], hi_ap[:, do, hs])
            nc.sync.dma_start(t[:, 1, do, :], lo_ap[:, do, hs])
        return t

    HT_W = H2W // P  # h tiles per weight half

    with tc.tile_pool(name="wpool", bufs=3) as wp:
        # k projection: first weight half, then the first x chunk, then the
        # remaining weight halves (so the first psum group starts ~sooner)
        wkc = [load_whalf(wp, "wk0", io["wkh"], io["wkl"], 0)]
        kfirst = load_x_chunk(io["xkh"], io["xkl"], 0)
        wkc += [load_whalf(wp, f"wk{h}", io["wkh"], io["wkl"], h)
                for h in range(1, NWH)]
        if pair:
            proj_hilo(wkc, io["xkh"], io["xkl"], None, None,
                      SK_OWN, dram_out=k_own, first_x=kfirst)
            nc.gpsimd.collective_compute(
                "AllGather", mybir.AluOpType.bypass,
                replica_groups=groups,
                ins=[k_own[:]], outs=[k_gath[:]])
            for r in range(2):
                nc.sync.dma_start(kth[:, :, r * SK_OWN:(r + 1) * SK_OWN],
                                  k_gath[r, :, 0])
                nc.sync.dma_start(ktl[:, :, r * SK_OWN:(r + 1) * SK_OWN],
                                  k_gath[r, :, 1])
        else:
            proj_hilo(wkc, io["xkh"], io["xkl"], kth, ktl, S,
                      first_x=kfirst)

        # v projection (f32r), v natural [sk, h] -> DRAM stage
        wv_h = []
        for h in range(NWH):
            wvt = wp.tile([P, DT, H2W], fr, name=f"wv{h}", tag="w", bufs=3)
            for do in range(DT):
                nc.sync.dma_start(wvt[:, do, :],
                                  io["wv"][:, do, h * H2W:(h + 1) * H2W])
            wv_h.append(wvt)
        for sko in range(SKO_OWN):
            xvt = strm.tile([P, DT, P], fr, name="xvt", tag="xs", bufs=4)
            nc.sync.dma_start(xvt[:], io["xv"][:, :, sko * P:(sko + 1) * P])
            v_dst = v_own if pair else v_stage
            for hc in range(HC):
                wvs = wv_h[hc * NB // H2W]
                ws = slice((hc * NB) % H2W, (hc * NB) % H2W + NB)
                pt = ps.tile([P, NB], f32, tag="mm", name="pv", bufs=2)
                for do in range(DT):
                    nc.tensor.matmul(pt[:], xvt[:, do, :], wvs[:, do, ws],
                                     start=(do == 0), stop=(do == DT - 1))
                vt = strm.tile([P, NB], fr, name="vt", tag="vo", bufs=2)
                nc.vector.tensor_copy(vt[:], pt[:])
                nc.sync.dma_start(v_dst[sko, :, hc * NB:(hc + 1) * NB],
                                  vt[:])
        if pair:
            nc.gpsimd.collective_compute(
                "AllGather", mybir.AluOpType.bypass,
                replica_groups=groups,
                ins=[v_own[:]], outs=[v_stage[:]])

        # q projection
        wqc = [load_whalf(wp, f"wq{h}", io["wqh"], io["wql"], h)
               for h in range(NWH)]
        proj_hilo(wqc, io["xqh"], io["xql"], qth, qtl, SQ)

    # ---------------- phase 3: scores + softmax + transpose ----------------
    attp_cm = tc.tile_pool(name="attp", bufs=1)
    attp = attp_cm.__enter__()
    attT = attp.tile([P, SKT, SQ], fr, name="attT")
    with tc.tile_pool(name="soft", bufs=2) as soft:
        for sqt in range(SQT):
            sq0 = sqt * P
            qs = slice(sq0, sq0 + P)
            EW = min(S, 2 * NB)  # columns per e tile
            es_ = [strm.tile([P, EW], fr, name="e", tag="xs", bufs=4)
                   for _ in range(S // EW)]
            nm_arr = soft.tile([P, SKC], f32, name="nm_arr")
            es_arr = soft.tile([P, SKC], f32, name="es_arr")
            for c in range(SKC):
                cs = slice(c * NB, (c + 1) * NB)
                sch = ps.tile([P, NB], f32, tag="sc", name="sch", bufs=4)
                for ho in range(HT):
                    nc.tensor.matmul(sch[:], qth[:, ho, qs], kth[:, ho, cs],
                                     start=(ho == 0), stop=False)
                    nc.tensor.matmul(sch[:], qth[:, ho, qs], ktl[:, ho, cs],
                                     start=False, stop=False)
                    nc.tensor.matmul(sch[:], qtl[:, ho, qs], kth[:, ho, cs],
                                     start=False, stop=(ho == HT - 1))
                nc.vector.tensor_tensor(sch[:], sch[:], colbias[:, cs],
                                        ALU.add)
                nc.vector.reduce_max(out=nm_arr[:, c:c + 1], in_=sch[:],
                                     axis=mybir.AxisListType.X, negate=True)
                # e_c = exp(s - m_c): frees this PSUM bank immediately
                ei = es_[(c * NB) // EW]
                ecs = slice((c * NB) % EW, (c * NB) % EW + NB)
                nc.scalar.activation(ei[:, ecs], sch[:], AF.Exp,
                                     bias=nm_arr[:, c:c + 1],
                                     accum_out=es_arr[:, c:c + 1])
            # global max and per-quarter rescale factors
            nmax = soft.tile([P, 1], f32, name="nmax")
            nc.vector.tensor_reduce(out=nmax[:], in_=nm_arr[:],
                                    op=ALU.min, axis=mybir.AxisListType.X)
            dm = soft.tile([P, SKC], f32, name="dm")
            nc.vector.tensor_scalar_sub(dm[:], nm_arr[:], nmax[:])
            fq = soft.tile([P, SKC], f32, name="fq")
            nc.scalar.activation(fq[:], dm[:], AF.Exp, scale=-1.0)
            wsum = soft.tile([P, SKC], f32, name="wsum")
            nc.vector.tensor_tensor(wsum[:], fq[:], es_arr[:], ALU.mult)
            esum = soft.tile([P, 1], f32, name="esum")
            nc.vector.reduce_sum(out=esum[:], in_=wsum[:],
                                 axis=mybir.AxisListType.X)
            recip = soft.tile([P, 1], f32, name="recip")
            nc.vector.reciprocal(recip[:], esum[:])
            r_arr = soft.tile([P, SKC], f32, name="r_arr")
            nc.vector.tensor_scalar_mul(r_arr[:], fq[:], recip[:])
            for c in range(SKC):
                ei = es_[(c * NB) // EW]
                ecs = slice((c * NB) % EW, (c * NB) % EW + NB)
                nc.vector.tensor_scalar_mul(ei[:, ecs], ei[:, ecs],
                                            r_arr[:, c:c + 1])
            for sko in range(SKT):
                ei = es_[(sko * P) // EW]
                ecs = slice((sko * P) % EW, (sko * P) % EW + P)
                ptr = ps.tile([P, P], fr, tag="tr", name="ptr", bufs=2)
                nc.tensor.transpose(ptr[:], ei[:, ecs], ident[:])
                nc.vector.tensor_copy(attT[:, sko, sq0:sq0 + P], ptr[:])

    # ---------------- phase 4: attend (v SBUF-resident) ----------------
    # vlo/vhi/attendedT rotate into the slots freed by kth/ktl/qc
    vlo = big.tile([P, SKT, H2], fr, name="vlo", tag="big")
    vhi = None
    if H2 < H:
        vhi = big.tile([P, SKT, H - H2], fr, name="vhi", tag="big")
    # column-major loads: attend consumes v ho-block by ho-block, so the
    # first attend group only waits for one 1MB load instead of all 8MB
    for ho in range(HT):
        h0 = ho * P
        dst = (vlo[:, :, h0:h0 + P] if h0 < H2
               else vhi[:, :, h0 - H2:h0 - H2 + P])
        nc.sync.dma_start(
            dst, v_stage[:, :, h0:h0 + P].rearrange("s p h -> p s h"))
    attendedT = big.tile([P, HT, SQ], fr, name="attendedT", tag="big")
    for sqc in range(NSQC):
        ss = slice(sqc * SQC, (sqc + 1) * SQC)
        for ho in range(HT):
            h0 = ho * P
            vt_src = (vlo[:, :, h0:h0 + P] if h0 < H2
                      else vhi[:, :, h0 - H2:h0 - H2 + P])
            pa = ps.tile([P, SQC], f32, tag="mm", name="pa", bufs=2)
            for sko in range(SKT):
                nc.tensor.matmul(pa[:], vt_src[:, sko, :], attT[:, sko, ss],
                                 start=(sko == 0), stop=(sko == SKT - 1))
            nc.vector.tensor_scalar_add(attendedT[:, ho, ss], pa[:],
                                        bv_t[:, ho, None])
    attp_cm.__exit__(None, None, None)

    # ---------------- phase 5: output projection ----------------
    # wd/bd live in the persistent stream tags (no new pool) so outproj can
    # interleave with the attend phase instead of waiting for pool turnover.
    bd_h = []
    for i in range(DEP // NB):
        bdt = strm.tile([P, NB], f32, name=f"bd{i}", tag="xs", bufs=4)
        nc.sync.dma_start(bdt[:], io["bd"][:, i * NB:(i + 1) * NB])
        bd_h.append(bdt)
    HW2 = HT // 2 if HT >= 2 else HT
    for dc in range(DC):
        ds_ = slice(dc * NB, (dc + 1) * NB)
        wd_t = []
        for g in range(HT // HW2):
            wdt = strm.tile([P, HW2, NB], fr, name=f"wd{g}", tag="xs",
                            bufs=4)
            nc.sync.dma_start(
                wdt[:], io["wd"][:, g * HW2:(g + 1) * HW2, ds_])
            wd_t.append(wdt)
        for sqt in range(SQT):
            sq0 = sqt * P
            po = ps.tile([P, NB], f32, tag="mm", name="po", bufs=2)
            for ho in range(HT):
                nc.tensor.matmul(
                    po[:], attendedT[:, ho, sq0:sq0 + P],
                    wd_t[ho // HW2][:, ho % HW2, :],
                    start=(ho == 0), stop=(ho == HT - 1))
            ot = strm.tile([P, NB], f32, name="ot", tag="vo", bufs=2)
            nc.vector.tensor_tensor(ot[:], po[:], bd_h[dc], ALU.add)
            nc.sync.dma_start(io["out"][sq0:sq0 + P, ds_], ot[:])


# ======================= host side =======================

def _split_hilo(x):
    hi = x.astype(BF16)
    lo = (x - hi.astype(np.float32)).astype(BF16)
    return hi, lo


def _to_pdt(x, inner=P):
    """[K, N] with K = KT*P -> [P, KT, N] (partition-major tiling)."""
    K, N = x.shape
    return np.ascontiguousarray(x.reshape(K // inner, inner, N).transpose(1, 0, 2))


def prep_core_inputs(query_c, keys_b, values_b, Wq, bq, Wk, Wv, Wd, bd,
                     colbias_b):
    """Build the per-core input map (numpy) from full f32 arrays."""
    out = {}
    qT = np.ascontiguousarray(query_c.T)       # [D, SQ]
    kT = np.ascontiguousarray(keys_b.T)        # [D, S]
    vT = np.ascontiguousarray(values_b.T)      # [D, S]
    for nm, arr in (("xq", qT), ("xk", kT)):
        hi, lo = _split_hilo(arr)
        out[nm + "h"] = _to_pdt(hi)
        out[nm + "l"] = _to_pdt(lo)
    out["xv"] = _to_pdt(vT)
    for nm, w in (("wq", Wq), ("wk", Wk)):
        hi, lo = _split_hilo(w)
        out[nm + "h"] = _to_pdt(hi)
        out[nm + "l"] = _to_pdt(lo)
    out["wv"] = _to_pdt(Wv)
    out["wd"] = _to_pdt(Wd)
    out["bd"] = np.ascontiguousarray(np.broadcast_to(bd.astype(np.float32), (P, bd.shape[0])))
    out["colbias"] = np.ascontiguousarray(np.broadcast_to(colbias_b.astype(np.float32), (P, colbias_b.shape[0])))
    return out


def build_program(S=S, D=D, H=H, DEP=DEP, SQ=SQ, num_devices=NCORES,
                  repeats=1, pair=False):
    from contextlib import ExitStack
    nc = bacc.Bacc("TRN2", target_bir_lowering=False, debug=False,
                   num_devices=num_devices)
    io = {}
    for name, (shape, dtp) in input_specs(S, D, H, DEP, SQ, pair).items():
        io[name] = nc.dram_tensor(name, shape, dtp, kind="ExternalInput").ap()
    io["out"] = nc.dram_tensor("out", [SQ, DEP], f32,
                               kind="ExternalOutput").ap()
    with tile.TileContext(nc) as tc:
        for _ in range(repeats):
            with ExitStack() as ctx:
                emit_attention(ctx, tc, io, S, D, H, DEP, SQ, pair=pair)
    nc.compile()
    return nc


_CACHE = {}


def kernel(query, keys, values, Wq, bq, Wk, bk, Wv, bv, Wd, bd):
    query = np.asarray(query, np.float32)
    keys = np.asarray(keys, np.float32)
    values = np.asarray(values, np.float32)
    Wq = np.asarray(Wq, np.float32)
    Wk = np.asarray(Wk, np.float32)
    Wv = np.asarray(Wv, np.float32)
    Wd = np.asarray(Wd, np.float32)
    bq = np.asarray(bq, np.float32)
    bv = np.asarray(bv, np.float32)
    bd = np.asarray(bd, np.float32)

    if "nc" not in _CACHE:
        _CACHE["nc"] = build_program()
    nc = _CACHE["nc"]

    # colbias[b, sk] = keys[b] @ (Wk @ bq), exact in f64
    wkbq = (Wk.astype(np.float64) @ bq.astype(np.float64)).astype(np.float32)
    colbias = keys @ wkbq  # [B, S]

    bv_tile = np.ascontiguousarray(
        bv.reshape(H // P, P).T).astype(np.float32)  # [P, HT]

    # weights / biases are identical for every core: prep once
    shared = {}
    for nm, w in (("wq", Wq), ("wk", Wk)):
        hi, lo = _split_hilo(w)
        shared[nm + "h"] = _to_pdt(hi)
        shared[nm + "l"] = _to_pdt(lo)
    shared["wv"] = _to_pdt(Wv)
    shared["wd"] = _to_pdt(Wd)
    shared["bd"] = np.ascontiguousarray(
        np.broadcast_to(bd.astype(np.float32), (P, DEP)))
    shared["bv"] = bv_tile

    # per-batch tensors (shared by the two q-half cores of a batch)
    batch_part = []
    for b in range(B):
        m = {}
        kT = np.ascontiguousarray(keys[b].T)
        vT = np.ascontiguousarray(values[b].T)
        hi, lo = _split_hilo(kT)
        m["xkh"] = _to_pdt(hi)
        m["xkl"] = _to_pdt(lo)
        m["xv"] = _to_pdt(vT)
        m["colbias"] = np.ascontiguousarray(
            np.broadcast_to(colbias[b], (P, S)))
        batch_part.append(m)

    in_maps = []
    for c in range(NCORES):
        b, qh = divmod(c, 2)
        qT = np.ascontiguousarray(query[b, qh * SQ:(qh + 1) * SQ].T)
        hi, lo = _split_hilo(qT)
        m = {"xqh": _to_pdt(hi), "xql": _to_pdt(lo)}
        m.update(batch_part[b])
        m.update(shared)
        in_maps.append(m)

    outs = _run_spmd(nc, in_maps)

    out = np.empty((B, S, DEP), np.float32)
    for c in range(NCORES):
        b, qh = divmod(c, 2)
        out[b, qh * SQ:(qh + 1) * SQ] = outs[c]
    return out


def _get_runner(nc):
    """Build (once) a cached jitted shard_map executor for nc."""
    if "runner" in _CACHE:
        return _CACHE["runner"]
    import jax
    import concourse.mybir as mybir_
    from concourse import bass2jax
    from concourse.bass2jax import _bass_exec_p, install_neuronx_cc_hook
    from jax.experimental.shard_map import shard_map
    from jax.sharding import Mesh, PartitionSpec

    install_neuronx_cc_hook()
    in_names, out_names, out_avals, zero_outs = [], [], [], []
    for alloc in nc.m.functions[0].allocations:
        if not isinstance(alloc, mybir_.MemoryLocationSet):
            continue
        name = alloc.memorylocations[0].name
        if alloc.kind == "ExternalInput":
            if nc.partition_id_tensor is None or \
                    name != nc.partition_id_tensor.name:
                in_names.append(name)
        elif alloc.kind == "ExternalOutput":
            out_names.append(name)
            shape = tuple(alloc.tensor_shape)
            dtp = mybir_.dt.np(alloc.dtype)
            out_avals.append(jax.core.ShapedArray(shape, dtp))
            zero_outs.append(np.zeros(shape, dtp))
    n_params = len(in_names)
    n_outs = len(out_avals)
    all_names = in_names + out_names
    pname = nc.partition_id_tensor.name if nc.partition_id_tensor else None
    if pname is not None:
        all_names = all_names + [pname]
    donate = tuple(range(n_params, n_params + n_outs))

    def _body(*args):
        operands = list(args)
        if pname is not None:
            operands.append(bass2jax.partition_id_tensor())
        outs = _bass_exec_p.bind(
            *operands,
            out_avals=tuple(out_avals),
            in_names=tuple(all_names),
            out_names=tuple(out_names),
            lowering_input_output_aliases=(),
            sim_require_finite=True,
            sim_require_nnan=True,
            nc=nc,
        )
        return tuple(outs)

    devices = jax.devices()[:NCORES]
    mesh = Mesh(np.asarray(devices), ("core",))
    in_specs = (PartitionSpec("core"),) * (n_params + n_outs)
    out_specs = (PartitionSpec("core"),) * n_outs
    sharded = jax.jit(
        shard_map(_body, mesh=mesh, in_specs=in_specs, out_specs=out_specs,
                  check_rep=False),
        donate_argnums=donate, keep_unused=True)
    runner = (sharded, in_names, out_names, zero_outs)
    _CACHE["runner"] = runner
    return runner


def _run_spmd(nc, in_maps):
    """Run nc on NCORES devices; returns list of per-core 'out' arrays."""
    sharded, in_names, out_names, zero_outs = _get_runner(nc)
    concat_in = [
        np.concatenate([np.asarray(m[name]) for m in in_maps], axis=0)
        for name in in_names
    ]
    concat_zeros = [
        np.zeros((NCORES * z.shape[0], *z.shape[1:]), z.dtype)
        for z in zero_outs
    ]
    out_arrs = sharded(*concat_in, *concat_zeros)
    oi = out_names.index("out")
    full = np.asarray(out_arrs[oi])
    per = full.reshape(NCORES, full.shape[0] // NCORES, *full.shape[1:])
    return [per[c] for c in range(NCORES)]



# revision 4
# speedup vs baseline: 5.0793x; 5.0793x over previous
"""Trainium2 Bass kernel for nn_Attention_47493748359201.

Single-head attention: q/k/v projections -> softmax(q k^T) v -> output proj.
Full shapes: query/keys/values [4, 2048, 1024], weights [1024, 1024].

Sharding: 8 cores = (batch, query-half). Each core computes the full K/V
projection for its batch plus its own 1024-row query slice; no collectives.

Precision (v2): single-pass f32r matmuls everywhere (11-bit mantissa).
Score abs error ~8e-3 -> softmax rel err ~1e-2, well under the 2e-2 gate.
Attention weights, v, attended and the output projection run in bf16
(values bounded, rel err ~2e-3 each).

Bias handling (exact):
  - bk: drops out of softmax (constant along the key axis).
  - bq: folded into q during the q-projection PSUM eviction (per-partition
    add), so scores = (q+bq).k directly.
  - bv: softmax rows sum to 1, so attended += bv (per-partition add during
    the attend PSUM eviction).
  - bd: partition-broadcast add during the output PSUM eviction.
"""
import sys

sys.path.insert(0, "/opt/trn_rl_repo")

import numpy as np
import ml_dtypes

import concourse.bass as bass
import concourse.mybir as mybir
import concourse.tile as tile
from concourse import bacc
from concourse.masks import make_identity

P = 128
NB = 512  # matmul moving free dim (one PSUM bank of f32)
AF = mybir.ActivationFunctionType
ALU = mybir.AluOpType
dt = mybir.dt
f32 = dt.float32
f32r = dt.float32r
bf16 = dt.bfloat16
BF16 = ml_dtypes.bfloat16

# full-problem constants
B, S, D, H, DEP = 4, 2048, 1024, 1024, 1024
NCORES = 8
SQ = B * S // NCORES  # 1024 query rows per core


def input_specs():
    """name -> (shape, mybir dtype) for the per-core DRAM inputs."""
    DT, HT = D // P, H // P
    return {
        "xq": ([P, DT, SQ], f32r),
        "xk": ([P, DT, S], f32r),
        "xv": ([P, DT, S], f32r),
        "wq": ([P, DT, H], f32r),
        "wk": ([P, DT, H], f32r),
        "wv": ([P, DT, H], f32r),
        "wd": ([P, HT, DEP], bf16),
        "bq": ([P, H // P], f32),
        "bv": ([P, H // P], f32),
        "bd": ([P, DEP], f32),
    }


def emit_attention(ctx, tc, io):
    """Emit the per-core attention program. io: dict name -> bass.AP
    (input_specs() names plus "out" [SQ, DEP] f32)."""
    nc = tc.nc
    DT, HT, SKT, SQT = D // P, H // P, S // P, SQ // P
    SKC = S // NB          # score/key column chunks
    HC = H // NB           # h chunks
    DC = DEP // NB         # output dep chunks
    SH = S // 2            # keys per kth tile (two big-pool slots)
    SQC = NB               # attend rhs chunk
    NSQC = SQ // SQC
    H2 = H // 2            # v low/high split point

    # ---------------- resident SBUF (whole kernel) ----------------
    res = ctx.enter_context(tc.tile_pool(name="res", bufs=1))
    ident_f = res.tile([P, P], f32)
    make_identity(nc, ident_f[:])
    ident = res.tile([P, P], f32r)
    nc.vector.tensor_copy(ident[:], ident_f[:])
    bq_t = res.tile([P, HT], f32)
    nc.sync.dma_start(bq_t[:], io["bq"])
    bv_t = res.tile([P, HT], f32)
    nc.sync.dma_start(bv_t[:], io["bv"])

    # big rotating slots (32KB/partition each, bufs=3):
    #   kth0, kth1, qc -> vlo, vhi, attendedT
    big = ctx.enter_context(tc.tile_pool(name="big", bufs=3))
    kth0 = big.tile([P, HT, SH], f32r, tag="big")
    kth1 = big.tile([P, HT, SH], f32r, tag="big")
    qc = big.tile([P, HT, SQ], f32r, tag="big")

    def kth(ho, c):
        """moving k operand for score chunk c (NB cols)."""
        t = kth0 if c * NB < SH else kth1
        off = c * NB - (0 if c * NB < SH else SH)
        return t[:, ho, off:off + NB]

    # DRAM scratch for v (natural [sk, h] layout, tiled [SKT, P, H], bf16)
    dram = ctx.enter_context(tc.tile_pool(name="dram", bufs=1, space="DRAM"))
    v_stage = dram.tile([SKT, P, H], bf16)

    ps = ctx.enter_context(tc.tile_pool(name="ps", bufs=1, space="PSUM"))
    # stream pool: persistent tags -> DMA prefetch crosses phase boundaries
    strm = ctx.enter_context(tc.tile_pool(name="strm", bufs=1))

    # ---------------- phases 0-2: projections ----------------
    def load_x_chunk(x_ap, c):
        cs = slice(c * NB, (c + 1) * NB)
        xt = strm.tile([P, DT, NB], f32r, name="xt", tag="xs", bufs=2)
        for do in range(DT):
            nc.sync.dma_start(xt[:, do, :], x_ap[:, do, cs])
        return xt

    def load_w(nm, w_ap, half):
        t = strm.tile([P, DT, H2], f32r, name=nm, tag="w", bufs=2)
        hs = slice(half * H2, (half + 1) * H2)
        for do in range(DT):
            nc.sync.dma_start(t[:, do, :], w_ap[:, do, hs])
        return t

    HT_W = H2 // P  # h tiles per weight half

    def proj(w_halves, x_ap, out_tiles, ncols, bias=None):
        """out[h, col] = W^T @ x (+bias per h-partition), single f32r pass.
        out_tiles: list of (tile, col0) covering ncols."""
        for c in range(ncols // NB):
            xt = load_x_chunk(x_ap, c)
            for ho in range(HT):
                pt = ps.tile([P, NB], f32, tag="mm", name="pt", bufs=2)
                w_t = w_halves[ho // HT_W]
                hs = slice((ho % HT_W) * P, (ho % HT_W + 1) * P)
                for do in range(DT):
                    nc.tensor.matmul(pt[:], w_t[:, do, hs], xt[:, do, :],
                                     start=(do == 0), stop=(do == DT - 1))
                ot, col0 = None, 0
                for t, c0 in out_tiles:
                    if c0 <= c * NB < c0 + t.shape[-1]:
                        ot, col0 = t, c0
                        break
                cs = slice(c * NB - col0, c * NB - col0 + NB)
                if bias is None:
                    nc.scalar.activation(ot[:, ho, cs], pt[:], AF.Copy)
                else:
                    nc.scalar.activation(ot[:, ho, cs], pt[:], AF.Identity,
                                         bias=bias[:, ho:ho + 1])

    # k projection: weights first, then stream x
    wk_h = [load_w(f"wk{h}", io["wk"], h) for h in range(2)]
    proj(wk_h, io["xk"], [(kth0, 0), (kth1, SH)], S)

    # v projection (stationary x, moving w) -> natural [sk, h] bf16 tiles
    wv_h = [load_w(f"wv{h}", io["wv"], h) for h in range(2)]
    for c in range(SKC):
        xvt = load_x_chunk(io["xv"], c)
        for kt in range(NB // P):
            sko = c * (NB // P) + kt
            ks = slice(kt * P, (kt + 1) * P)
            for hc in range(HC):
                wvs = wv_h[hc * NB // H2]
                ws = slice((hc * NB) % H2, (hc * NB) % H2 + NB)
                pt = ps.tile([P, NB], f32, tag="mm", name="pv", bufs=2)
                for do in range(DT):
                    nc.tensor.matmul(pt[:], xvt[:, do, ks], wvs[:, do, ws],
                                     start=(do == 0), stop=(do == DT - 1))
                vt = strm.tile([P, NB], bf16, name="vt", tag="vo", bufs=2)
                nc.vector.tensor_copy(vt[:], pt[:])
                nc.sync.dma_start(v_stage[sko, :, hc * NB:(hc + 1) * NB],
                                  vt[:])

    # q projection (+bq), f32r out
    wq_h = [load_w(f"wq{h}", io["wq"], h) for h in range(2)]
    proj(wq_h, io["xq"], [(qc, 0)], SQ, bias=bq_t)

    # ---------------- phase 3: scores + softmax + transpose ----------------
    attp_cm = tc.tile_pool(name="attp", bufs=1)
    attp = attp_cm.__enter__()
    attT = attp.tile([P, SKT, SQ], bf16, name="attT")
    with tc.tile_pool(name="soft", bufs=2) as soft:
        for sqt in range(SQT):
            sq0 = sqt * P
            qs = slice(sq0, sq0 + P)
            EW = 2 * NB  # columns per e tile
            es_ = [strm.tile([P, EW], f32r, name="e", tag="es", bufs=2)
                   for _ in range(S // EW)]
            nm_arr = soft.tile([P, SKC], f32, name="nm_arr")
            es_arr = soft.tile([P, SKC], f32, name="es_arr")
            for c in range(SKC):
                sch = ps.tile([P, NB], f32, tag="sc", name="sch", bufs=4)
                for ho in range(HT):
                    nc.tensor.matmul(sch[:], qc[:, ho, qs], kth(ho, c),
                                     start=(ho == 0), stop=(ho == HT - 1))
                nc.vector.reduce_max(out=nm_arr[:, c:c + 1], in_=sch[:],
                                     axis=mybir.AxisListType.X, negate=True)
                # e_c = exp(s - m_c): frees this PSUM bank immediately
                ei = es_[(c * NB) // EW]
                ecs = slice((c * NB) % EW, (c * NB) % EW + NB)
                nc.scalar.activation(ei[:, ecs], sch[:], AF.Exp,
                                     bias=nm_arr[:, c:c + 1],
                                     accum_out=es_arr[:, c:c + 1])
            # global max and per-quarter rescale factors
            nmax = soft.tile([P, 1], f32, name="nmax")
            nc.vector.tensor_reduce(out=nmax[:], in_=nm_arr[:],
                                    op=ALU.min, axis=mybir.AxisListType.X)
            dm = soft.tile([P, SKC], f32, name="dm")
            nc.vector.tensor_scalar_sub(dm[:], nm_arr[:], nmax[:])
            fq = soft.tile([P, SKC], f32, name="fq")
            nc.scalar.activation(fq[:], dm[:], AF.Exp, scale=-1.0)
            wsum = soft.tile([P, SKC], f32, name="wsum")
            nc.vector.tensor_tensor(wsum[:], fq[:], es_arr[:], ALU.mult)
            esum = soft.tile([P, 1], f32, name="esum")
            nc.vector.reduce_sum(out=esum[:], in_=wsum[:],
                                 axis=mybir.AxisListType.X)
            recip = soft.tile([P, 1], f32, name="recip")
            nc.vector.reciprocal(recip[:], esum[:])
            r_arr = soft.tile([P, SKC], f32, name="r_arr")
            nc.vector.tensor_scalar_mul(r_arr[:], fq[:], recip[:])
            for c in range(SKC):
                ei = es_[(c * NB) // EW]
                ecs = slice((c * NB) % EW, (c * NB) % EW + NB)
                nc.vector.tensor_scalar_mul(ei[:, ecs], ei[:, ecs],
                                            r_arr[:, c:c + 1])
            for sko in range(SKT):
                ei = es_[(sko * P) // EW]
                ecs = slice((sko * P) % EW, (sko * P) % EW + P)
                ptr = ps.tile([P, P], f32r, tag="tr", name="ptr", bufs=2)
                nc.tensor.transpose(ptr[:], ei[:, ecs], ident[:])
                nc.vector.tensor_copy(attT[:, sko, sq0:sq0 + P], ptr[:])

    # ---------------- phase 4: attend (v SBUF-resident, bf16) ----------------
    # vlo/vhi/attendedT rotate into the slots freed by kth0/kth1/qc
    vlo = big.tile([P, SKT, H2], bf16, name="vlo", tag="big")
    vhi = big.tile([P, SKT, H - H2], bf16, name="vhi", tag="big")
    # column-major loads: attend consumes v ho-block by ho-block, so the
    # first attend group only waits for one load instead of all of v
    for ho in range(HT):
        h0 = ho * P
        dst = (vlo[:, :, h0:h0 + P] if h0 < H2
               else vhi[:, :, h0 - H2:h0 - H2 + P])
        nc.sync.dma_start(
            dst, v_stage[:, :, h0:h0 + P].rearrange("s p h -> p s h"))
    attendedT = big.tile([P, HT, SQ], bf16, name="attendedT", tag="big")
    for sqc in range(NSQC):
        ss = slice(sqc * SQC, (sqc + 1) * SQC)
        for ho in range(HT):
            h0 = ho * P
            vt_src = (vlo[:, :, h0:h0 + P] if h0 < H2
                      else vhi[:, :, h0 - H2:h0 - H2 + P])
            pa = ps.tile([P, SQC], f32, tag="mm", name="pa", bufs=2)
            for sko in range(SKT):
                nc.tensor.matmul(pa[:], vt_src[:, sko, :], attT[:, sko, ss],
                                 start=(sko == 0), stop=(sko == SKT - 1))
            nc.scalar.activation(attendedT[:, ho, ss], pa[:], AF.Identity,
                                 bias=bv_t[:, ho:ho + 1])
    attp_cm.__exit__(None, None, None)

    # ---------------- phase 5: output projection ----------------
    # wd/bd live in the persistent stream tags (no new pool) so outproj can
    # interleave with the attend phase instead of waiting for pool turnover.
    bd_h = []
    for i in range(DC):
        bdt = strm.tile([P, NB], f32, name=f"bd{i}", tag="es", bufs=2)
        nc.sync.dma_start(bdt[:], io["bd"][:, i * NB:(i + 1) * NB])
        bd_h.append(bdt)
    for dc in range(DC):
        ds_ = slice(dc * NB, (dc + 1) * NB)
        wdt = strm.tile([P, HT, NB], bf16, name=f"wd{dc}", tag="xs", bufs=2)
        nc.sync.dma_start(wdt[:], io["wd"][:, :, ds_])
        for sqt in range(SQT):
            sq0 = sqt * P
            po = ps.tile([P, NB], f32, tag="mm", name="po", bufs=2)
            for ho in range(HT):
                nc.tensor.matmul(
                    po[:], attendedT[:, ho, sq0:sq0 + P], wdt[:, ho, :],
                    start=(ho == 0), stop=(ho == HT - 1))
            ot = strm.tile([P, NB], f32, name="ot", tag="vo", bufs=2)
            nc.vector.tensor_tensor(ot[:], po[:], bd_h[dc], ALU.add)
            nc.sync.dma_start(io["out"][sq0:sq0 + P, ds_], ot[:])


# ======================= host side =======================

def _to_pdt(x, dtype=np.float32):
    """[K, N] with K = KT*P -> [P, KT, N] (partition-major tiling)."""
    K, N = x.shape
    return np.ascontiguousarray(
        x.reshape(K // P, P, N).transpose(1, 0, 2).astype(dtype))


def prep_in_maps(query, keys, values, Wq, bq, Wk, bk, Wv, bv, Wd, bd):
    """Build the per-core input maps (numpy) from full f32 arrays."""
    query = np.asarray(query, np.float32)
    keys = np.asarray(keys, np.float32)
    values = np.asarray(values, np.float32)

    # weights / biases are identical for every core: prep once
    shared = {
        "wq": _to_pdt(np.asarray(Wq, np.float32)),
        "wk": _to_pdt(np.asarray(Wk, np.float32)),
        "wv": _to_pdt(np.asarray(Wv, np.float32)),
        "wd": _to_pdt(np.asarray(Wd, np.float32), BF16),
        "bq": np.ascontiguousarray(
            np.asarray(bq, np.float32).reshape(H // P, P).T),
        "bv": np.ascontiguousarray(
            np.asarray(bv, np.float32).reshape(H // P, P).T),
        "bd": np.ascontiguousarray(
            np.broadcast_to(np.asarray(bd, np.float32), (P, DEP))),
    }

    # per-batch tensors (shared by the two q-half cores of a batch)
    batch_part = []
    for b in range(B):
        batch_part.append({
            "xk": _to_pdt(np.ascontiguousarray(keys[b].T)),
            "xv": _to_pdt(np.ascontiguousarray(values[b].T)),
        })

    in_maps = []
    for c in range(NCORES):
        b, qh = divmod(c, 2)
        m = {"xq": _to_pdt(
            np.ascontiguousarray(query[b, qh * SQ:(qh + 1) * SQ].T))}
        m.update(batch_part[b])
        m.update(shared)
        in_maps.append(m)
    return in_maps


def build_program(num_devices=NCORES, repeats=1):
    from contextlib import ExitStack
    nc = bacc.Bacc("TRN2", target_bir_lowering=False, debug=False,
                   num_devices=num_devices)
    io = {}
    for name, (shape, dtp) in input_specs().items():
        io[name] = nc.dram_tensor(name, shape, dtp, kind="ExternalInput").ap()
    io["out"] = nc.dram_tensor("out", [SQ, DEP], f32,
                               kind="ExternalOutput").ap()
    with tile.TileContext(nc) as tc:
        for _ in range(repeats):
            with ExitStack() as ctx:
                emit_attention(ctx, tc, io)
    nc.compile()
    return nc


_CACHE = {}


def kernel(query, keys, values, Wq, bq, Wk, bk, Wv, bv, Wd, bd):
    if "nc" not in _CACHE:
        _CACHE["nc"] = build_program()
    nc = _CACHE["nc"]

    in_maps = prep_in_maps(query, keys, values, Wq, bq, Wk, bk, Wv, bv,
                           Wd, bd)
    outs = _run_spmd(nc, in_maps)

    out = np.empty((B, S, DEP), np.float32)
    for c in range(NCORES):
        b, qh = divmod(c, 2)
        out[b, qh * SQ:(qh + 1) * SQ] = outs[c]
    return out


def _get_runner(nc):
    """Build (once) a cached jitted shard_map executor for nc."""
    if "runner" in _CACHE:
        return _CACHE["runner"]
    import jax
    import concourse.mybir as mybir_
    from concourse import bass2jax
    from concourse.bass2jax import _bass_exec_p, install_neuronx_cc_hook
    from jax.experimental.shard_map import shard_map
    from jax.sharding import Mesh, PartitionSpec

    install_neuronx_cc_hook()
    in_names, out_names, out_avals, zero_outs = [], [], [], []
    for alloc in nc.m.functions[0].allocations:
        if not isinstance(alloc, mybir_.MemoryLocationSet):
            continue
        name = alloc.memorylocations[0].name
        if alloc.kind == "ExternalInput":
            if nc.partition_id_tensor is None or \
                    name != nc.partition_id_tensor.name:
                in_names.append(name)
        elif alloc.kind == "ExternalOutput":
            out_names.append(name)
            shape = tuple(alloc.tensor_shape)
            dtp = mybir_.dt.np(alloc.dtype)
            out_avals.append(jax.core.ShapedArray(shape, dtp))
            zero_outs.append(np.zeros(shape, dtp))
    n_params = len(in_names)
    n_outs = len(out_avals)
    all_names = in_names + out_names
    pname = nc.partition_id_tensor.name if nc.partition_id_tensor else None
    if pname is not None:
        all_names = all_names + [pname]
    donate = tuple(range(n_params, n_params + n_outs))

    def _body(*args):
        operands = list(args)
        if pname is not None:
            operands.append(bass2jax.partition_id_tensor())
        outs = _bass_exec_p.bind(
            *operands,
            out_avals=tuple(out_avals),
            in_names=tuple(all_names),
            out_names=tuple(out_names),
            lowering_input_output_aliases=(),
            sim_require_finite=True,
            sim_require_nnan=True,
            nc=nc,
        )
        return tuple(outs)

    devices = jax.devices()[:NCORES]
    mesh = Mesh(np.asarray(devices), ("core",))
    in_specs = (PartitionSpec("core"),) * (n_params + n_outs)
    out_specs = (PartitionSpec("core"),) * n_outs
    sharded = jax.jit(
        shard_map(_body, mesh=mesh, in_specs=in_specs, out_specs=out_specs,
                  check_rep=False),
        donate_argnums=donate, keep_unused=True)
    runner = (sharded, in_names, out_names, zero_outs)
    _CACHE["runner"] = runner
    return runner


def _run_spmd(nc, in_maps):
    """Run nc on NCORES devices; returns list of per-core 'out' arrays."""
    sharded, in_names, out_names, zero_outs = _get_runner(nc)
    concat_in = [
        np.concatenate([np.asarray(m[name]) for m in in_maps], axis=0)
        for name in in_names
    ]
    concat_zeros = [
        np.zeros((NCORES * z.shape[0], *z.shape[1:]), z.dtype)
        for z in zero_outs
    ]
    out_arrs = sharded(*concat_in, *concat_zeros)
    oi = out_names.index("out")
    full = np.asarray(out_arrs[oi])
    per = full.reshape(NCORES, full.shape[0] // NCORES, *full.shape[1:])
    return [per[c] for c in range(NCORES)]
